# revision 24
# baseline (speedup 1.0000x reference)
"""Trainium2 Bass kernel for nn_DecomposeModel (gated 2-layer MLP decompose).

Strategy:
  - Host: sort rows by group. Only group==0 rows need the left GateNN,
    only group==1 rows need the right GateNN, group==2 rows output zero.
    Deal g0/g1 rows round-robin across the 8 cores (data parallel), pad
    each segment to a fixed per-core cap so all cores run one SPMD program.
  - Device: activations kept transposed [feature, row] so every matmul
    uses the weights in native [in, out] layout as the stationary operand
    (out = W_tile.T @ xT_tile). bf16 matmuls, f32 PSUM accumulation,
    tanh/sigmoid on ScalarE with fused bias, gating product on VectorE.
    Input x-stream DMAs ride the Sync HWDGE ring, weights ride the
    Scalar HWDGE ring, outputs ride GpSimd SWDGE; explicit tile deps
    hold prefetches back so the first matmul gates on ~3 MB of DMA.
    Layer-1 runs all four tanh-path f-tiles before the sigmoid-path
    f-tiles so the wl1g weight load stays off the critical path.
  - Host: scatter device outputs back to full [B, H] f32 (g2 rows stay 0).

Optional env KERNEL_MODE: "bf16" (default, absmax err ~4e-3 vs f32
reference, HW ~197 us on 8 cores), "fp8g" (sigmoid-gate path of layer 1
in fp8 DoubleRow, err ~1e-2), "fp8" (all of layer 1 in fp8 DoubleRow,
~140 us, err ~1.5e-2).
"""

import os
import sys

try:
    import concourse  # noqa: F401
except ImportError:
    sys.path.insert(0, "/opt/trn_rl_repo")

import numpy as np
import ml_dtypes

import concourse.tile as tile
from concourse import bacc, mybir
from concourse.bass_utils import run_bass_kernel_spmd

B = 32768
H = 512
NCORES = 8
BC = B // NCORES  # per-core shard of the mask output
DEFAULT_CAP = 1408  # per-core per-branch row capacity (B/3/8 = 1365.3 avg)
# KERNEL_MODE: "bf16" (safest), "fp8g" (layer-1 sigmoid-gate path in fp8
# DoubleRow — low-sensitivity path, ~12%% faster), "fp8" (all of layer-1 in
# fp8 DoubleRow, ~30%% faster, ~4x bf16 error).
MODE = os.environ.get("KERNEL_MODE", "bf16")

BF16 = mybir.dt.bfloat16
F32 = mybir.dt.float32
I32 = mybir.dt.int32

# biases stacked [8, 512] in this order
BIAS_ORDER = ["bl1h", "bl1g", "bl2h", "bl2g", "br1h", "br1g", "br2h", "br2g"]

_PROGRAM_CACHE = {}
LAST_RESULT = None  # BassKernelResults of the most recent kernel() call


def _blocks(cap):
    out = []
    rem = cap
    while rem > 0:
        b = 512 if rem >= 576 else rem
        out.append(b)
        rem -= b
    return out


def build_program(cap0, cap1, mode="bf16"):
    """Emit + compile the SPMD program for per-branch caps (cap0, cap1)."""
    nc = bacc.Bacc("TRN2", target_bir_lowering=False, debug=False,
                   num_devices=NCORES)

    FP8 = mybir.dt.float8e4
    fp8_l1 = mode == "fp8"
    fp8_g = mode == "fp8g"
    L1DT = FP8 if fp8_l1 else BF16      # dtype of the h-path / main x
    GDT = FP8 if (fp8_l1 or fp8_g) else BF16  # dtype of the g-path
    # fp8: gate weights pre-scaled by 64 on the host so they sit in e4m3's
    # normal range (raw values ~±0.026 are subnormal); descale is free via
    # the activation scale field.
    h_scale = (1.0 / 64.0) if fp8_l1 else 1.0
    g_scale = (1.0 / 64.0) if (fp8_l1 or fp8_g) else 1.0
    n_first_chunks = 4 if mode == "bf16" else 2
    ncols = cap0 + cap1
    x1t_d = nc.dram_tensor("x1t", [3 * H, ncols], L1DT, kind="ExternalInput")
    x8_d = (nc.dram_tensor("x1t8", [3 * H, ncols], FP8, kind="ExternalInput")
            if fp8_g else None)
    lt_d = nc.dram_tensor("lt", [H, cap1], BF16, kind="ExternalInput")
    grp_d = nc.dram_tensor("grp", [BC], I32, kind="ExternalInput")
    bias_d = nc.dram_tensor("bias8", [8, H], F32, kind="ExternalInput")

    w_shapes = [("wl1h", 3 * H), ("wl1g", 3 * H),
                ("wl2h", H), ("wl2g", H),
                ("wr1h", 3 * H), ("wr1g", 3 * H),
                ("wr2h", 2 * H), ("wr2g", 2 * H)]
    def _wdt(name):
        if name in ("wl1h", "wr1h"):
            return L1DT
        if name in ("wl1g", "wr1g"):
            return GDT
        return BF16

    w_d = {}
    for name, kdim in w_shapes:
        w_d[name] = nc.dram_tensor(name, [kdim, H], _wdt(name),
                                   kind="ExternalInput")

    outt_d = nc.dram_tensor("outt", [H, ncols], F32, kind="ExternalOutput")
    fin_d = nc.dram_tensor("fin", [BC], I32, kind="ExternalOutput")

    with tile.TileContext(nc) as tc:
        with (
            tc.tile_pool(name="wsb", bufs=1) as wpool,
            tc.tile_pool(name="bsb", bufs=1) as bpool,
            tc.tile_pool(name="xsb", bufs=3) as xpool,
            tc.tile_pool(name="hsb", bufs=2) as hpool,
            tc.tile_pool(name="act", bufs=3) as apool,
            tc.tile_pool(name="acth", bufs=6) as ahpool,
            tc.tile_pool(name="osb", bufs=6) as opool,
            tc.tile_pool(name="msc", bufs=1) as mpool,
            tc.tile_pool(name="ps", bufs=8, space="PSUM") as pspool,
        ):
            # --- persistent weights / biases -------------------------------
            # Weights ride the Scalar HWDGE ring; the x-stream, outputs
            # and mask ride the Sync ring (GpSimd stays idle so its
            # expensive SWDGE tail drain disappears). wl1h and x-block-0
            # are split into kt-quarters and block 0 runs its tanh-path
            # matmuls kt-outer, so PE starts after ~0.8 MB of DMA and each
            # arriving quarter feeds a full 12-matmul burst; explicit deps
            # keep later prefetches from stealing HBM bandwidth during
            # that window.
            b_sb = bpool.tile([128, 8, 4], F32, tag="bias8")
            bias_ap = {n: b_sb[:, i, :] for i, n in enumerate(BIAS_ORDER)}

            # wl1h is split in two kt-halves so the very first matmuls can
            # start after ~1.5 MB instead of ~3 MB of DMA.
            w_sb = {}      # name -> list of (tile, kt_start, nkt)
            w_dma = {}     # name -> last dma instruction
            for name, kdim in w_shapes:
                nk = kdim // 128
                wdt = _wdt(name)
                if name in ("wl1h", "wl1g"):
                    nch = n_first_chunks if name == "wl1h" else 2
                    csz = nk // nch
                    lst = []
                    for ci in range(nch):
                        t = wpool.tile([128, csz, H], wdt,
                                       tag=f"w_{name}_{ci}")
                        lst.append((t, ci * csz, (ci + 1) * csz))
                    w_sb[name] = lst
                else:
                    t = wpool.tile([128, nk, H], wdt, tag=f"w_{name}")
                    w_sb[name] = [(t, 0, nk)]

            def _load_w(name):
                src = w_d[name].rearrange("(kt p) f -> p kt f", p=128)
                for t, k0, k1 in w_sb[name]:
                    w_dma[name] = nc.scalar.dma_start(
                        out=t[:], in_=src[:, k0:k1, :])
                return w_dma[name]

            def _wsl(name, kt, ft, span=1):
                for t, k0, k1 in w_sb[name]:
                    if k0 <= kt and kt + span <= k1:
                        if span == 1:
                            return t[:, kt - k0, ft * 128:(ft + 1) * 128]
                        return t[:, kt - k0:kt - k0 + span,
                                 ft * 128:(ft + 1) * 128]
                raise AssertionError

            _d_wl1h = _load_w("wl1h")
            nc.scalar.dma_start(
                out=b_sb[:], in_=bias_d.rearrange("b (j p) -> p b j", p=128)
            )

            x1t_r = x1t_d.rearrange("(kt p) n -> p kt n", p=128)
            x8_r = (x8_d.rearrange("(kt p) n -> p kt n", p=128)
                    if fp8_g else None)
            lt_r = lt_d.rearrange("(kt p) n -> p kt n", p=128)
            outt_r = outt_d.rearrange("(ft p) n -> p ft n", p=128)

            def branch(col0, cap, w1h, w1g, b1h, b1g, w2h, w2g, b2h, b2g,
                       with_lt, deferred_w=(), split_first=False):
                deferred_w = list(deferred_w)
                c0 = 0
                first = True
                for rblk in _blocks(cap):
                    cs = slice(col0 + c0, col0 + c0 + rblk)
                    if split_first and first:
                        csz = 12 // n_first_chunks
                        xparts = []
                        for ci in range(n_first_chunks):
                            xt = xpool.tile([128, csz, rblk], L1DT,
                                            tag=f"xq{ci}")
                            dxl = nc.sync.dma_start(
                                out=xt[:],
                                in_=x1t_r[:, ci * csz:(ci + 1) * csz, cs])
                            xparts.append((xt, ci * csz, (ci + 1) * csz))
                        _st["x0_dma"] = dxl
                    else:
                        x_sb = xpool.tile([128, 12, rblk], L1DT, tag="x")
                        dxl = nc.sync.dma_start(out=x_sb[:],
                                                in_=x1t_r[:, :, cs])
                        if _st.get("hold_x"):
                            tile.add_dep_helper(
                                dxl.ins, _st["hold_x"].ins, sync=True,
                                reason="defer x prefetch behind wl1h load")
                        xparts = [(x_sb, 0, 12)]
                    if fp8_g:
                        x8_sb = xpool.tile([128, 12, rblk], FP8, tag="x8")
                        dx8 = nc.sync.dma_start(out=x8_sb[:],
                                                in_=x8_r[:, :, cs])
                        if _st.get("hold_x") and not first:
                            tile.add_dep_helper(
                                dx8.ins, _st["hold_x"].ins, sync=True,
                                reason="defer x8 prefetch")
                    first = False

                    def _xsl(kt, span=1):
                        for t, k0, k1 in xparts:
                            if k0 <= kt and kt + span <= k1:
                                if span == 1:
                                    return t[:, kt - k0, :]
                                return t[:, kt - k0:kt - k0 + span, :]
                        raise AssertionError

                    while deferred_w:
                        wname = deferred_w.pop(0)
                        dw = _load_w(wname)
                        if wname == "wl1g":
                            _st["hold_x"] = dw

                    h_sb = hpool.tile([128, 4, rblk], BF16, tag="h")
                    hstep = 2 if fp8_l1 else 1
                    gstep = 2 if (fp8_l1 or fp8_g) else 1
                    hmode = (mybir.MatmulPerfMode.DoubleRow if hstep == 2
                             else None)
                    gmode = (mybir.MatmulPerfMode.DoubleRow if gstep == 2
                             else None)

                    def _gxsl(kt, span):
                        if fp8_g:
                            if span == 1:
                                return x8_sb[:, kt, :]
                            return x8_sb[:, kt:kt + span, :]
                        return _xsl(kt, span)

                    kt_outer = split_first and c0 == 0
                    phs, ths = [], []
                    if kt_outer:
                        # first block: x/wl1h arrive in kt-chunks; kt-outer
                        # order keeps PE fed by each chunk as it lands.
                        for ft in range(4):
                            ph = pspool.tile([128, rblk], F32, tag="ps")
                            phs.append(ph)
                        for kt in range(0, 12, hstep):
                            for ft in range(4):
                                nc.tensor.matmul(
                                    phs[ft][:], _wsl(w1h, kt, ft, hstep),
                                    _xsl(kt, hstep),
                                    start=(kt == 0),
                                    stop=(kt == 12 - hstep),
                                    perf_mode=hmode,
                                )
                        for ft in range(4):
                            th = ahpool.tile([128, rblk], BF16, tag="th")
                            nc.scalar.activation(
                                th[:], phs[ft][:],
                                mybir.ActivationFunctionType.Tanh,
                                bias=b1h[:, ft:ft + 1], scale=h_scale)
                            ths.append(th)
                    else:
                        for ft in range(4):
                            ph = pspool.tile([128, rblk], F32, tag="ps")
                            for kt in range(0, 12, hstep):
                                nc.tensor.matmul(
                                    ph[:], _wsl(w1h, kt, ft, hstep),
                                    _xsl(kt, hstep),
                                    start=(kt == 0), stop=(kt == 12 - hstep),
                                    perf_mode=hmode,
                                )
                            th = ahpool.tile([128, rblk], BF16, tag="th")
                            nc.scalar.activation(
                                th[:], ph[:],
                                mybir.ActivationFunctionType.Tanh,
                                bias=b1h[:, ft:ft + 1], scale=h_scale)
                            ths.append(th)
                    if kt_outer:
                        pgs = []
                        for ft in range(4):
                            pg = pspool.tile([128, rblk], F32, tag="ps")
                            pgs.append(pg)
                        for kt in range(0, 12, gstep):
                            for ft in range(4):
                                nc.tensor.matmul(
                                    pgs[ft][:], _wsl(w1g, kt, ft, gstep),
                                    _gxsl(kt, gstep),
                                    start=(kt == 0),
                                    stop=(kt == 12 - gstep),
                                    perf_mode=gmode,
                                )
                        for ft in range(4):
                            sg = apool.tile([128, rblk], BF16, tag="sg")
                            nc.scalar.activation(
                                sg[:], pgs[ft][:],
                                mybir.ActivationFunctionType.Sigmoid,
                                bias=b1g[:, ft:ft + 1], scale=g_scale)
                            nc.vector.tensor_mul(h_sb[:, ft, :],
                                                 ths[ft][:], sg[:])
                    else:
                        for ft in range(4):
                            pg = pspool.tile([128, rblk], F32, tag="ps")
                            for kt in range(0, 12, gstep):
                                nc.tensor.matmul(
                                    pg[:], _wsl(w1g, kt, ft, gstep),
                                    _gxsl(kt, gstep),
                                    start=(kt == 0), stop=(kt == 12 - gstep),
                                    perf_mode=gmode,
                                )
                            sg = apool.tile([128, rblk], BF16, tag="sg")
                            nc.scalar.activation(
                                sg[:], pg[:],
                                mybir.ActivationFunctionType.Sigmoid,
                                bias=b1g[:, ft:ft + 1], scale=g_scale)
                            nc.vector.tensor_mul(h_sb[:, ft, :],
                                                 ths[ft][:], sg[:])

                    if with_lt:
                        lt_sb = xpool.tile([128, 4, rblk], BF16, tag="ltx")
                        dlt = nc.sync.dma_start(
                            out=lt_sb[:],
                            in_=lt_r[:, :, c0: c0 + rblk],
                        )
                        if _st.get("hold_x"):
                            tile.add_dep_helper(
                                dlt.ins, _st["hold_x"].ins, sync=True,
                                reason="defer lt prefetch behind wl1h load")
                    nk2 = 8 if with_lt else 4
                    for ft in range(4):
                        ph = pspool.tile([128, rblk], F32, tag="ps")
                        for kt in range(nk2):
                            rhs = (h_sb[:, kt, :] if kt < 4
                                   else lt_sb[:, kt - 4, :])
                            nc.tensor.matmul(
                                ph[:], _wsl(w2h, kt, ft), rhs,
                                start=(kt == 0), stop=(kt == nk2 - 1),
                            )
                        pg = pspool.tile([128, rblk], F32, tag="ps")
                        for kt in range(nk2):
                            rhs = (h_sb[:, kt, :] if kt < 4
                                   else lt_sb[:, kt - 4, :])
                            nc.tensor.matmul(
                                pg[:], _wsl(w2g, kt, ft), rhs,
                                start=(kt == 0), stop=(kt == nk2 - 1),
                            )
                        th = apool.tile([128, rblk], F32, tag="th2")
                        sg = apool.tile([128, rblk], F32, tag="sg2")
                        nc.scalar.activation(
                            th[:], ph[:], mybir.ActivationFunctionType.Tanh,
                            bias=b2h[:, ft:ft + 1])
                        nc.scalar.activation(
                            sg[:], pg[:], mybir.ActivationFunctionType.Sigmoid,
                            bias=b2g[:, ft:ft + 1])
                        o_sb = opool.tile([128, rblk], F32, tag="o")
                        nc.vector.tensor_mul(o_sb[:], th[:], sg[:])
                        nc.sync.dma_start(
                            out=outt_r[:, ft, cs],
                            in_=o_sb[:],
                        )
                    c0 += rblk

            _st = {"hold_x": _d_wl1h}
            branch(0, cap0,
                   "wl1h", "wl1g", bias_ap["bl1h"], bias_ap["bl1g"],
                   "wl2h", "wl2g", bias_ap["bl2h"], bias_ap["bl2g"],
                   with_lt=False, deferred_w=["wl1g", "wl2h", "wl2g"],
                   split_first=True)
            branch(cap0, cap1,
                   "wr1h", "wr1g", bias_ap["br1h"], bias_ap["br1g"],
                   "wr2h", "wr2g", bias_ap["br2h"], bias_ap["br2g"],
                   with_lt=True,
                   deferred_w=["wr1h", "wr1g", "wr2h", "wr2g"])

            # --- finished mask (tiny, fully overlapped) --------------------
            gt = mpool.tile([128, BC // 128], I32, tag="grp")
            nc.sync.dma_start(
                out=gt[:], in_=grp_d.rearrange("(p j) -> p j", p=128)
            )
            ft_ = mpool.tile([128, BC // 128], I32, tag="fin")
            nc.vector.tensor_scalar(ft_[:], gt[:], 2, None,
                                    op0=mybir.AluOpType.is_equal)
            nc.sync.dma_start(
                out=fin_d.rearrange("(p j) -> p j", p=128), in_=ft_[:]
            )

    nc.compile()
    return nc


def _get_program(cap0, cap1, mode):
    key = (cap0, cap1, mode)
    if key not in _PROGRAM_CACHE:
        _PROGRAM_CACHE[key] = build_program(cap0, cap1, mode)
    return _PROGRAM_CACHE[key]


def _roundup(x, m):
    return ((x + m - 1) // m) * m


def kernel(node_hidden, node_context, label_embedding, left_embedding, group,
           Wl1h, bl1h, Wl1g, bl1g, Wl2h, bl2h, Wl2g, bl2g,
           Wr1h, br1h, Wr1g, br1g, Wr2h, br2h, Wr2g, br2g,
           trace=False, trace_kwargs=None):
    global LAST_RESULT
    group = np.asarray(group)
    idx0 = np.flatnonzero(group == 0)
    idx1 = np.flatnonzero(group == 1)
    per0 = [idx0[c::NCORES] for c in range(NCORES)]
    per1 = [idx1[c::NCORES] for c in range(NCORES)]
    need0 = max(len(p) for p in per0)
    need1 = max(len(p) for p in per1)
    cap0 = DEFAULT_CAP if need0 <= DEFAULT_CAP else _roundup(need0, 64)
    cap1 = DEFAULT_CAP if need1 <= DEFAULT_CAP else _roundup(need1, 64)

    mode = MODE
    fp8_l1 = mode == "fp8"
    fp8_g = mode == "fp8g"
    nc = _get_program(cap0, cap1, mode)

    bf = ml_dtypes.bfloat16
    f8 = ml_dtypes.float8_e4m3fn
    l1dt = f8 if fp8_l1 else bf
    gdt = f8 if (fp8_l1 or fp8_g) else bf
    xcat = np.concatenate(
        [np.asarray(node_hidden), np.asarray(node_context),
         np.asarray(label_embedding)], axis=1)  # [B, 3H] f32
    lemb = np.asarray(left_embedding)

    hsc = 64.0 if fp8_l1 else 1.0
    gsc = 64.0 if (fp8_l1 or fp8_g) else 1.0
    shared = {
        "wl1h": np.ascontiguousarray(np.asarray(Wl1h) * hsc).astype(l1dt),
        "wl1g": np.ascontiguousarray(np.asarray(Wl1g) * gsc).astype(gdt),
        "wr1h": np.ascontiguousarray(np.asarray(Wr1h) * hsc).astype(l1dt),
        "wr1g": np.ascontiguousarray(np.asarray(Wr1g) * gsc).astype(gdt),
        "wl2h": np.ascontiguousarray(Wl2h).astype(bf),
        "wl2g": np.ascontiguousarray(Wl2g).astype(bf),
        "wr2h": np.ascontiguousarray(Wr2h).astype(bf),
        "wr2g": np.ascontiguousarray(Wr2g).astype(bf),
        "bias8": np.ascontiguousarray(np.stack(
            [bl1h, bl1g, bl2h, bl2g, br1h, br1g, br2h, br2g]),
            dtype=np.float32),
    }

    in_maps = []
    for c in range(NCORES):
        rows0 = np.zeros(cap0, dtype=np.int64)
        rows0[:len(per0[c])] = per0[c]
        rows1 = np.zeros(cap1, dtype=np.int64)
        rows1[:len(per1[c])] = per1[c]
        rows = np.concatenate([rows0, rows1])
        x1tT = np.ascontiguousarray(xcat[rows].T)  # [3H, ncols] f32
        x1t = x1tT.astype(l1dt)
        lt = np.ascontiguousarray(lemb[rows1].T).astype(bf)  # [H, cap1]
        m = dict(shared)
        m["x1t"] = x1t
        if fp8_g:
            m["x1t8"] = x1tT.astype(f8)
        m["lt"] = lt
        m["grp"] = np.ascontiguousarray(group[c * BC:(c + 1) * BC],
                                        dtype=np.int32)
        in_maps.append(m)

    res = run_bass_kernel_spmd(nc, in_maps, list(range(NCORES)),
                               trace=trace, **(trace_kwargs or {}))
    LAST_RESULT = res

    children = np.zeros((B, H), dtype=np.float32)
    finished = np.empty(B, dtype=np.int32)
    for c in range(NCORES):
        outt = res.results[c]["outt"]  # [H, ncols] f32
        outr = outt.T  # [ncols, H]
        if len(per0[c]):
            children[per0[c]] = outr[:len(per0[c])]
        if len(per1[c]):
            children[per1[c]] = outr[cap0:cap0 + len(per1[c])]
        finished[c * BC:(c + 1) * BC] = res.results[c]["fin"]
    return children, finished


# revision 25
# speedup vs baseline: 1.0087x; 1.0087x over previous
"""Trainium2 Bass kernel for nn_DecomposeModel (gated 2-layer MLP decompose).

Strategy:
  - Host: sort rows by group. Only group==0 rows need the left GateNN,
    only group==1 rows need the right GateNN, group==2 rows output zero.
    Deal g0/g1 rows round-robin across the 8 cores (data parallel), pad
    each segment to a fixed per-core cap so all cores run one SPMD program.
  - Device: activations kept transposed [feature, row] so every matmul
    uses the weights in native [in, out] layout as the stationary operand
    (out = W_tile.T @ xT_tile). bf16 matmuls, f32 PSUM accumulation,
    tanh/sigmoid on ScalarE with fused bias, gating product on VectorE.
    Input x-stream DMAs ride the Sync HWDGE ring, weights ride the
    Scalar HWDGE ring, outputs ride GpSimd SWDGE; explicit tile deps
    hold prefetches back so the first matmul gates on ~3 MB of DMA.
    Layer-1 runs all four tanh-path f-tiles before the sigmoid-path
    f-tiles so the wl1g weight load stays off the critical path.
  - Host: scatter device outputs back to full [B, H] f32 (g2 rows stay 0).

Optional env KERNEL_MODE: "bf16" (default, absmax err ~4e-3 vs f32
reference, HW ~197 us on 8 cores), "fp8g" (sigmoid-gate path of layer 1
in fp8 DoubleRow, err ~1e-2), "fp8" (all of layer 1 in fp8 DoubleRow,
~140 us, err ~1.5e-2).
"""

import os
import sys

try:
    import concourse  # noqa: F401
except ImportError:
    sys.path.insert(0, "/opt/trn_rl_repo")

import numpy as np
import ml_dtypes

import concourse.tile as tile
from concourse import bacc, mybir
from concourse.bass_utils import run_bass_kernel_spmd

B = 32768
H = 512
NCORES = 8
BC = B // NCORES  # per-core shard of the mask output
DEFAULT_CAP = 1408  # per-core per-branch row capacity (B/3/8 = 1365.3 avg)
# KERNEL_MODE: "bf16" (safest), "fp8g" (layer-1 sigmoid-gate path in fp8
# DoubleRow — low-sensitivity path, ~12%% faster), "fp8" (all of layer-1 in
# fp8 DoubleRow, ~30%% faster, ~4x bf16 error).
MODE = os.environ.get("KERNEL_MODE", "bf16")

BF16 = mybir.dt.bfloat16
F32 = mybir.dt.float32
I32 = mybir.dt.int32

# biases stacked [8, 512] in this order
BIAS_ORDER = ["bl1h", "bl1g", "bl2h", "bl2g", "br1h", "br1g", "br2h", "br2g"]

_PROGRAM_CACHE = {}
LAST_RESULT = None  # BassKernelResults of the most recent kernel() call


def _blocks(cap):
    out = []
    rem = cap
    while rem > 0:
        b = 512 if rem >= 576 else rem
        out.append(b)
        rem -= b
    return out


def build_program(cap0, cap1, mode="bf16"):
    """Emit + compile the SPMD program for per-branch caps (cap0, cap1)."""
    nc = bacc.Bacc("TRN2", target_bir_lowering=False, debug=False,
                   num_devices=NCORES)

    FP8 = mybir.dt.float8e4
    fp8_l1 = mode == "fp8"
    fp8_g = mode == "fp8g"
    L1DT = FP8 if fp8_l1 else BF16      # dtype of the h-path / main x
    GDT = FP8 if (fp8_l1 or fp8_g) else BF16  # dtype of the g-path
    # fp8: gate weights pre-scaled by 64 on the host so they sit in e4m3's
    # normal range (raw values ~±0.026 are subnormal); descale is free via
    # the activation scale field.
    h_scale = (1.0 / 64.0) if fp8_l1 else 1.0
    g_scale = (1.0 / 64.0) if (fp8_l1 or fp8_g) else 1.0
    n_first_chunks = 4 if mode == "bf16" else 2
    ncols = cap0 + cap1
    x1t_d = nc.dram_tensor("x1t", [3 * H, ncols], L1DT, kind="ExternalInput")
    x8_d = (nc.dram_tensor("x1t8", [3 * H, ncols], FP8, kind="ExternalInput")
            if fp8_g else None)
    lt_d = nc.dram_tensor("lt", [H, cap1], BF16, kind="ExternalInput")
    grp_d = nc.dram_tensor("grp", [BC], I32, kind="ExternalInput")
    bias_d = nc.dram_tensor("bias8", [8, H], F32, kind="ExternalInput")

    w_shapes = [("wl1h", 3 * H), ("wl1g", 3 * H),
                ("wl2h", H), ("wl2g", H),
                ("wr1h", 3 * H), ("wr1g", 3 * H),
                ("wr2h", 2 * H), ("wr2g", 2 * H)]
    def _wdt(name):
        if name in ("wl1h", "wr1h"):
            return L1DT
        if name in ("wl1g", "wr1g"):
            return GDT
        return BF16

    w_d = {}
    for name, kdim in w_shapes:
        w_d[name] = nc.dram_tensor(name, [kdim, H], _wdt(name),
                                   kind="ExternalInput")

    outt_d = nc.dram_tensor("outt", [H, ncols], F32, kind="ExternalOutput")
    fin_d = nc.dram_tensor("fin", [BC], I32, kind="ExternalOutput")

    with tile.TileContext(nc) as tc:
        with (
            tc.tile_pool(name="wsb", bufs=1) as wpool,
            tc.tile_pool(name="bsb", bufs=1) as bpool,
            tc.tile_pool(name="xsb", bufs=3) as xpool,
            tc.tile_pool(name="hsb", bufs=2) as hpool,
            tc.tile_pool(name="act", bufs=3) as apool,
            tc.tile_pool(name="acth", bufs=6) as ahpool,
            tc.tile_pool(name="osb", bufs=6) as opool,
            tc.tile_pool(name="msc", bufs=1) as mpool,
            tc.tile_pool(name="ps", bufs=8, space="PSUM") as pspool,
        ):
            # --- persistent weights / biases -------------------------------
            # Weights ride the Scalar HWDGE ring; the x-stream, outputs
            # and mask ride the Sync ring (GpSimd stays idle so its
            # expensive SWDGE tail drain disappears). wl1h and x-block-0
            # are split into kt-quarters and block 0 runs its tanh-path
            # matmuls kt-outer, so PE starts after ~0.8 MB of DMA and each
            # arriving quarter feeds a full 12-matmul burst; explicit deps
            # keep later prefetches from stealing HBM bandwidth during
            # that window.
            b_sb = bpool.tile([128, 8, 4], F32, tag="bias8")
            bias_ap = {n: b_sb[:, i, :] for i, n in enumerate(BIAS_ORDER)}

            # wl1h is split in two kt-halves so the very first matmuls can
            # start after ~1.5 MB instead of ~3 MB of DMA.
            w_sb = {}      # name -> list of (tile, kt_start, nkt)
            w_dma = {}     # name -> last dma instruction
            for name, kdim in w_shapes:
                nk = kdim // 128
                wdt = _wdt(name)
                if name == "wl1h":
                    csz = nk // n_first_chunks
                    lst = []
                    for ci in range(n_first_chunks):
                        t = wpool.tile([128, csz, H], wdt,
                                       tag=f"w_wl1h_{ci}")
                        lst.append((t, ci * csz, (ci + 1) * csz))
                    w_sb[name] = lst
                else:
                    t = wpool.tile([128, nk, H], wdt, tag=f"w_{name}")
                    w_sb[name] = [(t, 0, nk)]

            def _load_w(name):
                src = w_d[name].rearrange("(kt p) f -> p kt f", p=128)
                for t, k0, k1 in w_sb[name]:
                    w_dma[name] = nc.scalar.dma_start(
                        out=t[:], in_=src[:, k0:k1, :])
                return w_dma[name]

            def _wsl(name, kt, ft, span=1):
                for t, k0, k1 in w_sb[name]:
                    if k0 <= kt and kt + span <= k1:
                        if span == 1:
                            return t[:, kt - k0, ft * 128:(ft + 1) * 128]
                        return t[:, kt - k0:kt - k0 + span,
                                 ft * 128:(ft + 1) * 128]
                raise AssertionError

            _d_wl1h = _load_w("wl1h")
            nc.scalar.dma_start(
                out=b_sb[:], in_=bias_d.rearrange("b (j p) -> p b j", p=128)
            )

            x1t_r = x1t_d.rearrange("(kt p) n -> p kt n", p=128)
            x8_r = (x8_d.rearrange("(kt p) n -> p kt n", p=128)
                    if fp8_g else None)
            lt_r = lt_d.rearrange("(kt p) n -> p kt n", p=128)
            outt_r = outt_d.rearrange("(ft p) n -> p ft n", p=128)

            def branch(col0, cap, w1h, w1g, b1h, b1g, w2h, w2g, b2h, b2g,
                       with_lt, deferred_w=(), split_first=False):
                deferred_w = list(deferred_w)
                c0 = 0
                first = True
                for rblk in _blocks(cap):
                    cs = slice(col0 + c0, col0 + c0 + rblk)
                    if split_first and first:
                        csz = 12 // n_first_chunks
                        xparts = []
                        for ci in range(n_first_chunks):
                            xt = xpool.tile([128, csz, rblk], L1DT,
                                            tag=f"xq{ci}")
                            dxl = nc.sync.dma_start(
                                out=xt[:],
                                in_=x1t_r[:, ci * csz:(ci + 1) * csz, cs])
                            xparts.append((xt, ci * csz, (ci + 1) * csz))
                        _st["x0_dma"] = dxl
                    else:
                        x_sb = xpool.tile([128, 12, rblk], L1DT, tag="x")
                        dxl = nc.sync.dma_start(out=x_sb[:],
                                                in_=x1t_r[:, :, cs])
                        if _st.get("hold_x"):
                            tile.add_dep_helper(
                                dxl.ins, _st["hold_x"].ins, sync=True,
                                reason="defer x prefetch behind wl1h load")
                        xparts = [(x_sb, 0, 12)]
                    if fp8_g:
                        x8_sb = xpool.tile([128, 12, rblk], FP8, tag="x8")
                        dx8 = nc.sync.dma_start(out=x8_sb[:],
                                                in_=x8_r[:, :, cs])
                        if _st.get("hold_x") and not first:
                            tile.add_dep_helper(
                                dx8.ins, _st["hold_x"].ins, sync=True,
                                reason="defer x8 prefetch")
                    first = False

                    def _xsl(kt, span=1):
                        for t, k0, k1 in xparts:
                            if k0 <= kt and kt + span <= k1:
                                if span == 1:
                                    return t[:, kt - k0, :]
                                return t[:, kt - k0:kt - k0 + span, :]
                        raise AssertionError

                    while deferred_w:
                        wname = deferred_w.pop(0)
                        dw = _load_w(wname)
                        if wname == "wl1g":
                            _st["hold_x"] = dw

                    h_sb = hpool.tile([128, 4, rblk], BF16, tag="h")
                    hstep = 2 if fp8_l1 else 1
                    gstep = 2 if (fp8_l1 or fp8_g) else 1
                    hmode = (mybir.MatmulPerfMode.DoubleRow if hstep == 2
                             else None)
                    gmode = (mybir.MatmulPerfMode.DoubleRow if gstep == 2
                             else None)

                    def _gxsl(kt, span):
                        if fp8_g:
                            if span == 1:
                                return x8_sb[:, kt, :]
                            return x8_sb[:, kt:kt + span, :]
                        return _xsl(kt, span)

                    kt_outer = split_first and c0 == 0
                    phs, ths = [], []
                    if kt_outer:
                        # first block: x/wl1h arrive in kt-chunks; kt-outer
                        # order keeps PE fed by each chunk as it lands.
                        for ft in range(4):
                            ph = pspool.tile([128, rblk], F32, tag="ps")
                            phs.append(ph)
                        for kt in range(0, 12, hstep):
                            for ft in range(4):
                                nc.tensor.matmul(
                                    phs[ft][:], _wsl(w1h, kt, ft, hstep),
                                    _xsl(kt, hstep),
                                    start=(kt == 0),
                                    stop=(kt == 12 - hstep),
                                    perf_mode=hmode,
                                )
                        for ft in range(4):
                            th = ahpool.tile([128, rblk], BF16, tag="th")
                            nc.scalar.activation(
                                th[:], phs[ft][:],
                                mybir.ActivationFunctionType.Tanh,
                                bias=b1h[:, ft:ft + 1], scale=h_scale)
                            ths.append(th)
                    else:
                        for ft in range(4):
                            ph = pspool.tile([128, rblk], F32, tag="ps")
                            for kt in range(0, 12, hstep):
                                nc.tensor.matmul(
                                    ph[:], _wsl(w1h, kt, ft, hstep),
                                    _xsl(kt, hstep),
                                    start=(kt == 0), stop=(kt == 12 - hstep),
                                    perf_mode=hmode,
                                )
                            th = ahpool.tile([128, rblk], BF16, tag="th")
                            nc.scalar.activation(
                                th[:], ph[:],
                                mybir.ActivationFunctionType.Tanh,
                                bias=b1h[:, ft:ft + 1], scale=h_scale)
                            ths.append(th)
                    for ft in range(4):
                        pg = pspool.tile([128, rblk], F32, tag="ps")
                        for kt in range(0, 12, gstep):
                            nc.tensor.matmul(
                                pg[:], _wsl(w1g, kt, ft, gstep),
                                _gxsl(kt, gstep),
                                start=(kt == 0), stop=(kt == 12 - gstep),
                                perf_mode=gmode,
                            )
                        sg = apool.tile([128, rblk], BF16, tag="sg")
                        nc.scalar.activation(
                            sg[:], pg[:], mybir.ActivationFunctionType.Sigmoid,
                            bias=b1g[:, ft:ft + 1], scale=g_scale)
                        nc.vector.tensor_mul(h_sb[:, ft, :], ths[ft][:], sg[:])

                    if with_lt:
                        lt_sb = xpool.tile([128, 4, rblk], BF16, tag="ltx")
                        dlt = nc.sync.dma_start(
                            out=lt_sb[:],
                            in_=lt_r[:, :, c0: c0 + rblk],
                        )
                        if _st.get("hold_x"):
                            tile.add_dep_helper(
                                dlt.ins, _st["hold_x"].ins, sync=True,
                                reason="defer lt prefetch behind wl1h load")
                    nk2 = 8 if with_lt else 4
                    for ft in range(4):
                        ph = pspool.tile([128, rblk], F32, tag="ps")
                        for kt in range(nk2):
                            rhs = (h_sb[:, kt, :] if kt < 4
                                   else lt_sb[:, kt - 4, :])
                            nc.tensor.matmul(
                                ph[:], _wsl(w2h, kt, ft), rhs,
                                start=(kt == 0), stop=(kt == nk2 - 1),
                            )
                        pg = pspool.tile([128, rblk], F32, tag="ps")
                        for kt in range(nk2):
                            rhs = (h_sb[:, kt, :] if kt < 4
                                   else lt_sb[:, kt - 4, :])
                            nc.tensor.matmul(
                                pg[:], _wsl(w2g, kt, ft), rhs,
                                start=(kt == 0), stop=(kt == nk2 - 1),
                            )
                        th = apool.tile([128, rblk], F32, tag="th2")
                        sg = apool.tile([128, rblk], F32, tag="sg2")
                        nc.scalar.activation(
                            th[:], ph[:], mybir.ActivationFunctionType.Tanh,
                            bias=b2h[:, ft:ft + 1])
                        nc.scalar.activation(
                            sg[:], pg[:], mybir.ActivationFunctionType.Sigmoid,
                            bias=b2g[:, ft:ft + 1])
                        o_sb = opool.tile([128, rblk], F32, tag="o")
                        nc.vector.tensor_mul(o_sb[:], th[:], sg[:])
                        nc.sync.dma_start(
                            out=outt_r[:, ft, cs],
                            in_=o_sb[:],
                        )
                    c0 += rblk

            _st = {"hold_x": _d_wl1h}
            branch(0, cap0,
                   "wl1h", "wl1g", bias_ap["bl1h"], bias_ap["bl1g"],
                   "wl2h", "wl2g", bias_ap["bl2h"], bias_ap["bl2g"],
                   with_lt=False, deferred_w=["wl1g", "wl2h", "wl2g"],
                   split_first=True)
            branch(cap0, cap1,
                   "wr1h", "wr1g", bias_ap["br1h"], bias_ap["br1g"],
                   "wr2h", "wr2g", bias_ap["br2h"], bias_ap["br2g"],
                   with_lt=True,
                   deferred_w=["wr1h", "wr1g", "wr2h", "wr2g"])

            # --- finished mask (tiny, fully overlapped) --------------------
            gt = mpool.tile([128, BC // 128], I32, tag="grp")
            nc.sync.dma_start(
                out=gt[:], in_=grp_d.rearrange("(p j) -> p j", p=128)
            )
            ft_ = mpool.tile([128, BC // 128], I32, tag="fin")
            nc.vector.tensor_scalar(ft_[:], gt[:], 2, None,
                                    op0=mybir.AluOpType.is_equal)
            nc.sync.dma_start(
                out=fin_d.rearrange("(p j) -> p j", p=128), in_=ft_[:]
            )

    nc.compile()
    return nc


def _get_program(cap0, cap1, mode):
    key = (cap0, cap1, mode)
    if key not in _PROGRAM_CACHE:
        _PROGRAM_CACHE[key] = build_program(cap0, cap1, mode)
    return _PROGRAM_CACHE[key]


def _roundup(x, m):
    return ((x + m - 1) // m) * m


def kernel(node_hidden, node_context, label_embedding, left_embedding, group,
           Wl1h, bl1h, Wl1g, bl1g, Wl2h, bl2h, Wl2g, bl2g,
           Wr1h, br1h, Wr1g, br1g, Wr2h, br2h, Wr2g, br2g,
           trace=False, trace_kwargs=None):
    global LAST_RESULT
    group = np.asarray(group)
    idx0 = np.flatnonzero(group == 0)
    idx1 = np.flatnonzero(group == 1)
    per0 = [idx0[c::NCORES] for c in range(NCORES)]
    per1 = [idx1[c::NCORES] for c in range(NCORES)]
    need0 = max(len(p) for p in per0)
    need1 = max(len(p) for p in per1)
    cap0 = DEFAULT_CAP if need0 <= DEFAULT_CAP else _roundup(need0, 64)
    cap1 = DEFAULT_CAP if need1 <= DEFAULT_CAP else _roundup(need1, 64)

    mode = MODE
    fp8_l1 = mode == "fp8"
    fp8_g = mode == "fp8g"
    nc = _get_program(cap0, cap1, mode)

    bf = ml_dtypes.bfloat16
    f8 = ml_dtypes.float8_e4m3fn
    l1dt = f8 if fp8_l1 else bf
    gdt = f8 if (fp8_l1 or fp8_g) else bf
    xcat = np.concatenate(
        [np.asarray(node_hidden), np.asarray(node_context),
         np.asarray(label_embedding)], axis=1)  # [B, 3H] f32
    lemb = np.asarray(left_embedding)

    hsc = 64.0 if fp8_l1 else 1.0
    gsc = 64.0 if (fp8_l1 or fp8_g) else 1.0
    shared = {
        "wl1h": np.ascontiguousarray(np.asarray(Wl1h) * hsc).astype(l1dt),
        "wl1g": np.ascontiguousarray(np.asarray(Wl1g) * gsc).astype(gdt),
        "wr1h": np.ascontiguousarray(np.asarray(Wr1h) * hsc).astype(l1dt),
        "wr1g": np.ascontiguousarray(np.asarray(Wr1g) * gsc).astype(gdt),
        "wl2h": np.ascontiguousarray(Wl2h).astype(bf),
        "wl2g": np.ascontiguousarray(Wl2g).astype(bf),
        "wr2h": np.ascontiguousarray(Wr2h).astype(bf),
        "wr2g": np.ascontiguousarray(Wr2g).astype(bf),
        "bias8": np.ascontiguousarray(np.stack(
            [bl1h, bl1g, bl2h, bl2g, br1h, br1g, br2h, br2g]),
            dtype=np.float32),
    }

    in_maps = []
    for c in range(NCORES):
        rows0 = np.zeros(cap0, dtype=np.int64)
        rows0[:len(per0[c])] = per0[c]
        rows1 = np.zeros(cap1, dtype=np.int64)
        rows1[:len(per1[c])] = per1[c]
        rows = np.concatenate([rows0, rows1])
        x1tT = np.ascontiguousarray(xcat[rows].T)  # [3H, ncols] f32
        x1t = x1tT.astype(l1dt)
        lt = np.ascontiguousarray(lemb[rows1].T).astype(bf)  # [H, cap1]
        m = dict(shared)
        m["x1t"] = x1t
        if fp8_g:
            m["x1t8"] = x1tT.astype(f8)
        m["lt"] = lt
        m["grp"] = np.ascontiguousarray(group[c * BC:(c + 1) * BC],
                                        dtype=np.int32)
        in_maps.append(m)

    res = run_bass_kernel_spmd(nc, in_maps, list(range(NCORES)),
                               trace=trace, **(trace_kwargs or {}))
    LAST_RESULT = res

    children = np.zeros((B, H), dtype=np.float32)
    finished = np.empty(B, dtype=np.int32)
    for c in range(NCORES):
        outt = res.results[c]["outt"]  # [H, ncols] f32
        outr = outt.T  # [ncols, H]
        if len(per0[c]):
            children[per0[c]] = outr[:len(per0[c])]
        if len(per1[c]):
            children[per1[c]] = outr[cap0:cap0 + len(per1[c])]
        finished[c * BC:(c + 1) * BC] = res.results[c]["fin"]
    return children, finished
